# revision 2
# baseline (speedup 1.0000x reference)
"""Trainium2 Bass kernel for nn_KManifoldClusterModel (moe_routing).

Computation (reference):
    c  = clip((C[ii])^2 / (rowsum((C[ii])^2) + 1e-8), 0.1/n, 1.0)   [B, n]
    v  = V[ii]                                                      [B, d, n]
    x_ = einsum('jkd,bdj->bkj', U, v)                               [B, D, n]
returns (x_, c, v).

Shapes: B=4096, n=64 clusters, d=16, D=512, N=200000.

Strategy: batch-shard over the 8 NeuronCores (512 batch rows per core).
The index gather (rows of C/V at ii) is a host-side data-distribution step;
each core receives its per-batch slices pre-transposed so the tiny d=16
contraction sits on the partition axis:
  - per-cluster matmul: lhsT = v_j^T [d=16, b=128], rhs = U_j^T [d=16, k],
    out = x_[b, k, j] tile [128, k] in PSUM.
  - clusters are striped across the four 32-row PE strips (tile_position)
    so 4 matmuls run concurrently.
  - PSUM tiles are evacuated by ScalarE/VectorE copies that interleave the
    cluster axis (stride 64) into an SBUF staging buffer laid out exactly
    like the [b, k, j] output, so the final HBM write is one contiguous
    32 KB-per-partition DMA.
"""

import sys

sys.path.insert(0, "/opt/trn_rl_repo")

import numpy as np

# ---- problem constants (hardcoded; kernel.py must be self-contained) ----
B = 4096  # batch
NCLUS = 64  # n clusters
DSMALL = 16  # d
DBIG = 512  # D
N_CORES = 8
BC = B // N_CORES  # 512 batch rows per core
NT = BC // 128  # 4 b-tiles of 128 per core
CLIP_LO = 0.1 / NCLUS
EPS = 1e-8

_CACHE = {}


def _build_program():
    """Build + schedule the per-core Bass program (SPMD; all cores identical)."""
    from concourse import bacc, tile
    import concourse.mybir as mybir

    f32 = mybir.dt.float32

    nc = bacc.Bacc(
        "TRN2", target_bir_lowering=False, debug=False, enable_asserts=False
    )

    # DRAM I/O (per-core shapes)
    vT = nc.dram_tensor("vT", [128, 8192], f32, kind="ExternalInput").ap()
    Ut = nc.dram_tensor("Ut", [128, 8192], f32, kind="ExternalInput").ap()
    cb = nc.dram_tensor("cb", [128, 2 * 128], f32, kind="ExternalInput").ap()
    x_out = nc.dram_tensor(
        "x_out", [BC, DBIG, NCLUS], f32, kind="ExternalOutput"
    ).ap()
    c_out = nc.dram_tensor("c_out", [BC, NCLUS], f32, kind="ExternalOutput").ap()

    with tile.TileContext(nc) as tc:
        with (
            tc.tile_pool(name="inp", bufs=1) as inp,
            tc.tile_pool(name="psum", bufs=2, space="PSUM") as psum,
            tc.tile_pool(name="stage", bufs=3) as stage,
            tc.tile_pool(name="cp", bufs=1) as cp,
        ):
            # ---- load inputs in 4 chunks each (group-range) for overlap ----
            vt_sb = []
            ut_sb = []
            for cidx in range(4):
                tv = inp.tile([128, 2048], f32, tag=f"vt{cidx}")
                nc.sync.dma_start(tv[:], vT[:, 2048 * cidx : 2048 * (cidx + 1)])
                vt_sb.append(tv)
                tu = inp.tile([128, 2048], f32, tag=f"ut{cidx}")
                nc.sync.dma_start(tu[:], Ut[:, 2048 * cidx : 2048 * (cidx + 1)])
                ut_sb.append(tu)

            cb_sb = inp.tile([128, NT * NCLUS], f32, tag="cb")
            nc.sync.dma_start(cb_sb[:], cb[:])

            # ---- c activation: sq/rowsum(sq), clipped ----
            sq = cp.tile([128, NT * NCLUS], f32, tag="sq")
            nc.vector.tensor_mul(sq[:], cb_sb[:], cb_sb[:])
            ssum = cp.tile([128, NT], f32, tag="ssum")
            nc.vector.reduce_sum(
                ssum.rearrange("p (t o) -> p t o", o=1)[:],
                sq.rearrange("p (t j) -> p t j", t=NT)[:],
                axis=mybir.AxisListType.X,
            )
            nc.vector.tensor_scalar_add(ssum[:], ssum[:], EPS)
            rcp = cp.tile([128, NT], f32, tag="rcp")
            nc.vector.reciprocal(rcp[:], ssum[:])
            cact = cp.tile([128, NT * NCLUS], f32, tag="cact")
            cact_v = cact.rearrange("p (t j) -> p t j", t=NT)
            sq_v = sq.rearrange("p (t j) -> p t j", t=NT)
            for t in range(NT):
                nc.vector.tensor_scalar(
                    cact_v[:, t, :],
                    sq_v[:, t, :],
                    rcp[:, t : t + 1],
                    None,
                    op0=mybir.AluOpType.mult,
                )
                nc.vector.tensor_scalar(
                    cact_v[:, t, :],
                    cact_v[:, t, :],
                    CLIP_LO,
                    1.0,
                    op0=mybir.AluOpType.max,
                    op1=mybir.AluOpType.min,
                )
                nc.sync.dma_start(
                    c_out[128 * t : 128 * (t + 1), :], cact_v[:, t, :]
                )

            # ---- main loop: x_ = per-cluster GEMMs ----
            for t in range(NT):  # b-tile of 128 batch rows
                for h in range(2):  # k-half: k in [256h, 256h+256)
                    st = [
                        stage.tile(
                            [128, 128 * NCLUS], f32, tag="stage", name=f"st{t}_{h}_{i}"
                        )
                        for i in range(2)
                    ]
                    for r in range(16):  # round: clusters 4r..4r+3
                        ps = psum.tile([128, 2048], f32)
                        g, gsub = divmod(r, 4)
                        for a in range(4):  # strip a <-> cluster j = 4r+a
                            lhsT = vt_sb[g][
                                32 * a : 32 * a + 16,
                                512 * gsub + 128 * t : 512 * gsub + 128 * (t + 1),
                            ]
                            rhs = ut_sb[g][
                                32 * a : 32 * a + 16,
                                512 * gsub + 256 * h : 512 * gsub + 256 * (h + 1),
                            ]
                            nc.tensor.matmul(
                                ps[:, 512 * a : 512 * a + 256],
                                lhsT,
                                rhs,
                                start=True,
                                stop=True,
                                tile_position=(32 * a, 0),
                            )
                        # evacuate PSUM -> staging with cluster interleave
                        ps_v = ps.rearrange("p (a k) -> p a k", a=4)
                        for qq in range(2):
                            src = ps_v[:, :, 128 * qq : 128 * (qq + 1)]
                            dst = st[qq].rearrange("p (k j) -> p j k", j=NCLUS)[
                                :, 4 * r : 4 * r + 4, :
                            ]
                            if (r + qq) % 2 == 0:
                                nc.scalar.copy(dst, src)
                            else:
                                nc.vector.tensor_copy(dst, src)
                    for qq in range(2):
                        q = 2 * h + qq
                        nc.sync.dma_start(
                            x_out[
                                128 * t : 128 * (t + 1),
                                128 * q : 128 * (q + 1),
                                :,
                            ],
                            st[qq].rearrange("p (k j) -> p k j", j=NCLUS)[:],
                        )

    nc.compile()
    return nc


def _pack_inputs(ii, C, V, U):
    """Host-side shard + layout. Returns (in_maps, v_full)."""
    v_full = V[ii]  # [B, 16, 64]
    cb_full = C[ii]  # [B, 64]

    # Ut[32s+d, g*512+k] = U[4g+s, k, d]   (same for every core)
    ut = np.zeros((128, 8192), dtype=np.float32)
    # U: [64, 512, 16] -> (g, s, k, d) -> (s, d, g, k)
    ut_v = U.reshape(16, 4, DBIG, DSMALL).transpose(1, 3, 0, 2).reshape(64, 8192)
    ut.reshape(4, 32, 8192)[:, :16, :] = ut_v.reshape(4, 16, 8192)

    in_maps = []
    for core in range(N_CORES):
        b0 = core * BC
        vc = v_full[b0 : b0 + BC]  # [512, 16, 64] (b, d, j)
        # vT[32s+d, g*512+b] = vc[b, d, 4g+s]
        vt = np.zeros((128, 8192), dtype=np.float32)
        vt_v = (
            vc.reshape(BC, DSMALL, 16, 4).transpose(3, 1, 2, 0).reshape(64, 8192)
        )
        vt.reshape(4, 32, 8192)[:, :16, :] = vt_v.reshape(4, 16, 8192)

        cc = cb_full[b0 : b0 + BC]  # [512, 64]
        cbm = (
            cc.reshape(NT, 128, NCLUS)
            .transpose(1, 0, 2)
            .reshape(128, NT * NCLUS)
            .astype(np.float32)
        )
        in_maps.append(
            {"vT": vt, "Ut": ut, "cb": np.ascontiguousarray(cbm)}
        )
    return in_maps, v_full


def kernel(ii, C, V, U):
    import concourse.bass_utils as bass_utils

    ii = np.asarray(ii).astype(np.int64)
    C = np.asarray(C, dtype=np.float32)
    V = np.asarray(V, dtype=np.float32)
    U = np.asarray(U, dtype=np.float32)

    if "nc" not in _CACHE:
        _CACHE["nc"] = _build_program()
    nc = _CACHE["nc"]

    in_maps, v_full = _pack_inputs(ii, C, V, U)
    res = bass_utils.run_bass_kernel_spmd(nc, in_maps, core_ids=list(range(N_CORES)))

    x_ = np.concatenate([res.results[i]["x_out"] for i in range(N_CORES)], axis=0)
    c = np.concatenate([res.results[i]["c_out"] for i in range(N_CORES)], axis=0)
    return (x_, c, v_full)
